# revision 9
# baseline (speedup 1.0000x reference)
"""Batch-parallel attention kernel for 8 Trainium2 NeuronCores.

Problem: out[b,x,h] = sum_y softmax_y(sum_h' k[b,x,h']*q[b,y,h']) * v[b,y,h]
with q,k,v: [16, 2048, 128] fp32.  This is standard attention with the roles
of q and k swapped (queries = k rows, keys = q rows), no 1/sqrt(H) scale.

Sharding: batch dim (16) across 8 cores (pure data parallel), 2 batches per
core; flash-style x/y block tiling within a core.

Per-core algorithm (per batch, per x-half of 1024 score columns):
  Host supplies qT/kT = q/k transposed to [H, S] so H=128 sits on SBUF
  partitions (v stays natural).  fp32 and f32r are bit-identical, so DMA
  loads land directly in f32r tiles (no DVE cast pass at all).
  For each y-block j (128 rows):
    sT_j[y, x]   = qT_j^T @ kT       (f32r matmuls, N=512, PSUM)
    eT_j         = exp(sT_j - 30)    (ScalarE, PSUM -> SBUF, BF16 out; the
                                      -30 shift widens overflow headroom and
                                      cancels exactly in the normalization)
    outT[h, x]  += v_j^T @ eT_j      (PSUM accumulate over all j; bf16
                                      moving operand, 1 cyc/row)
    acc2(p)      = eT_2p + eT_2p+1   (DVE pair-sums, bf16 at 2x rate)
    acc4(q)      = acc2 + acc2       (DVE quad-sums, bf16 at 2x rate)
  The softmax denominator l = ones^T @ acc4 runs on PE over only 4 tiles
  (8 small matmuls) instead of 18, deferred into the next loop's tail.
  No running-max subtraction is needed: scores are ~N(0, sqrt(128)) and the
  observed max ~84 stays far below the shifted overflow point (118.7).

Scheduling (in-order engine queues make emission order = execution order
per engine):
  - MM1(j) is emitted at iteration j; MM2(j) at iteration j+2, giving
    exp(j) a ~2-iteration window instead of zero slack.
  - ps_s has 3 PSUM slots (6 banks) + outT accumulator (2 banks) = 8.
    The l accumulator PSUM is gone (l-sums borrow a ps_s slot in the tail).
  - The first two MM1/exp of the next (b, xh) are emitted inside the last
    two iterations of the current one, so ACT never drains at boundaries.
  - Each (b, xh)'s tail is deferred into the next loop's iterations 3/7/11
    (l-sums + l copy; l transposes + out transposes 0-3 + scale; out
    transposes 4-7 + scale + store), borrowing ps_s slots so the
    DVE->PE dependency latency hides under real work.
  - A dummy-matmul chain + a dummy Exp at the start warm the PE HAM clock
    gate and preload the ACT table set while the first DMAs run.
"""
import os
import sys
import types
from contextlib import ExitStack

import numpy as np

import concourse.bass as bass
import concourse.tile as tile
from concourse import mybir
from concourse.bass_utils import run_bass_kernel_spmd
from concourse.masks import make_identity

F32 = mybir.dt.float32
F32R = mybir.dt.float32r
BF16 = mybir.dt.bfloat16
Act = mybir.ActivationFunctionType

B, S, H = 16, 2048, 128
NCORES = 8
BPC = B // NCORES  # batches per core
XH = 1024          # x-half width
NJ = S // 128      # y blocks


# ---------------------------------------------------------------------------
# Workaround: this walrus build rejects instructions carrying more than one
# semaphore wait ("Too many sync wait commands", seen on CTRL Drain and S3_LW
# Matmult).  Hoist all-but-one wait of every instruction onto wait-only
# EventSemaphore instructions on the same engine, inserted just before it.
_wsplit_counter = [0]


def _split_waits(nc, max_waits: int = 1):
    for func in nc.m.functions:
        for blk in func.blocks:
            insts = blk.instructions
            i = 0
            while i < len(insts):
                inst = insts[i]
                si = inst.sync_info
                waits = list(si.on_wait) if si is not None else []
                if len(waits) > max_waits:
                    keep = waits[-max_waits:]
                    hoist = waits[:-max_waits]
                    inst.sync_info = mybir.SyncInfo(
                        on_wait=keep, on_update=list(si.on_update)
                    )
                    new_insts = []
                    for w in hoist:
                        _wsplit_counter[0] += 1
                        ev = mybir.InstEventSemaphore(
                            name=f"WSPLIT-{_wsplit_counter[0]}", ins=[], outs=[]
                        )
                        ev.engine = inst.engine
                        ev.sync_info = mybir.SyncInfo(on_wait=[w], on_update=[])
                        new_insts.append(ev)
                    insts[i:i] = new_insts
                    i += len(new_insts)
                i += 1


# NTFF profiling shim: the axon .so supports NRT profiling but the antenv
# glue module is absent in this image; register it so trace=True works.
def _install_ntff_hook():
    if "antenv.axon_hooks" in sys.modules:
        return
    try:
        from trn_agent_boot.trn_boot import _ntff_profile_via_ctypes

        hook = _ntff_profile_via_ctypes("/opt/axon/libaxon_pjrt.so")
    except Exception:
        hook = None
    mod = types.ModuleType("antenv.axon_hooks")
    mod.get_axon_ntff_profile_hook = lambda: hook
    mod.set_axon_ntff_profile_hook = lambda h: None
    sys.modules["antenv.axon_hooks"] = mod


def _build():
    nc = bass.Bass("TRN2", target_bir_lowering=False, debug=False)
    qt = nc.dram_tensor("qt", [BPC, H, S], F32, kind="ExternalInput")
    kt = nc.dram_tensor("kt", [BPC, H, S], F32, kind="ExternalInput")
    v = nc.dram_tensor("v", [BPC, S, H], F32, kind="ExternalInput")
    out = nc.dram_tensor("out", [BPC, S, H], F32, kind="ExternalOutput")

    with tile.TileContext(nc) as tc, ExitStack() as ctx:
        consts = ctx.enter_context(tc.tile_pool(name="consts", bufs=1))
        raw = ctx.enter_context(tc.tile_pool(name="raw", bufs=4))
        qkv = ctx.enter_context(tc.tile_pool(name="qkv", bufs=2))
        et_pool = ctx.enter_context(tc.tile_pool(name="et", bufs=10))
        a2_pool = ctx.enter_context(tc.tile_pool(name="a2", bufs=4))
        a4_pool = ctx.enter_context(tc.tile_pool(name="a4", bufs=6))
        sb_small = ctx.enter_context(tc.tile_pool(name="sb_small", bufs=2))
        outs = ctx.enter_context(tc.tile_pool(name="outs", bufs=2))
        ps_s = ctx.enter_context(tc.tile_pool(name="ps_s", bufs=3, space="PSUM"))
        ps_o = ctx.enter_context(tc.tile_pool(name="ps_o", bufs=1, space="PSUM"))

        ident = consts.tile([128, 128], F32)
        make_identity(nc, ident[:])
        # touch Exp first thing so the ACT table set loads under the DMAs
        warm = consts.tile([128, 2], F32)
        nc.vector.memset(warm[:], 0.0)
        nc.scalar.activation(warm[:], warm[:], Act.Exp)
        exp_bias = consts.tile([128, 1], F32)
        nc.vector.memset(exp_bias[:], -30.0)
        ones_f = consts.tile([128, 2], F32)
        nc.vector.memset(ones_f[:], 1.0)
        ones_r = consts.tile([128, 2], F32R)
        nc.vector.tensor_copy(ones_r[:], ones_f[:])
        ones_bf = consts.tile([128, 2], BF16)
        nc.vector.tensor_copy(ones_bf[:], ones_f[:])
        # dummy matmul chain: keeps the PE busy during the initial DMAs so
        # the HAM clock-gate is at full rate when real matmuls arrive
        warm_z = consts.tile([128, 512], F32, tag="wz")
        nc.vector.memset(warm_z[:], 0.0)
        warm_r = consts.tile([128, 512], F32R)
        nc.vector.tensor_copy(warm_r[:], warm_z[:])
        ps_junk = ps_s.tile([128, XH], F32, tag="ps_s")
        for _ in range(10):
            nc.tensor.matmul(
                ps_junk[:, 0:512], warm_r[:, 0:128], warm_r[:], start=True, stop=True
            )
        junk_sb = consts.tile([128, 2], F32, tag="wjunk")
        nc.vector.tensor_copy(junk_sb[:], ps_junk[:, 0:2])

        def emit_loads(b, fine):
            # DMA straight into f32r tiles (bit-identical to f32), chunked
            # so compute starts early.  First batch uses finer leading
            # chunks to cut the startup serial path.
            qr = qkv.tile([128, S], F32R, tag="qr")
            kr = qkv.tile([128, S], F32R, tag="kr")
            vr = qkv.tile([128, S], BF16, tag="vr")

            def load_k(lo, n):
                if fine:
                    # startup path: sync-engine DMA + DVE cast runs in
                    # parallel with the gpsimd v loads (gpsimd SWDGE
                    # generation is serialized, ~1us per chunk)
                    t = raw.tile([128, n], F32, tag="raw")
                    nc.sync.dma_start(t[:], kt.ap()[b][:, bass.ds(lo, n)])
                    nc.vector.tensor_copy(kr[:, bass.ds(lo, n)], t[:])
                else:
                    nc.gpsimd.dma_start(
                        kr[:, bass.ds(lo, n)], kt.ap()[b][:, bass.ds(lo, n)]
                    )

            def load_q(lo, n):
                if fine:
                    t = raw.tile([128, n], F32, tag="raw")
                    nc.sync.dma_start(t[:], qt.ap()[b][:, bass.ds(lo, n)])
                    nc.vector.tensor_copy(qr[:, bass.ds(lo, n)], t[:])
                else:
                    nc.gpsimd.dma_start(
                        qr[:, bass.ds(lo, n)], qt.ap()[b][:, bass.ds(lo, n)]
                    )

            def load_v(lo, n):
                # v[b] rows [lo, lo+n) presented as [128p, (j 128h)];
                # gpsimd DMA casts f32 -> bf16 in flight (MM2 wants bf16
                # to match the bf16 eT moving operand)
                v_chunk = bass.AP(
                    tensor=v,
                    offset=b * S * H + lo * H,
                    ap=[[H, 128], [128 * H, n // 128], [1, H]],
                )
                nc.gpsimd.dma_start(vr[:, bass.ds(lo, n)], v_chunk)

            if fine:
                load_k(0, 512)
                load_q(0, 256)
                load_v(0, 256)
                load_k(512, 512)
                load_q(256, 768)
                load_v(256, 768)
                load_k(1024, 1024)
                load_q(1024, 1024)
                load_v(1024, 1024)
            else:
                for hc in range(2):
                    load_k(hc * XH, XH)
                    load_q(hc * XH, XH)
                    load_v(hc * XH, XH)
            return qr, kr, vr

        qkv_b = {0: emit_loads(0, fine=True)}

        # Tail work for iteration (b, xh) is deferred into the NEXT
        # iteration's j-loop so its DVE->PE dependency latency hides under
        # real work.  part0 (loop end): evacuate po on DVE.
        # part1 (next loop, it==3): l-sums over the 4 acc4 tiles into a
        #   stolen ps_s slot, l copied to SBUF.
        # part2 (next loop, it==7): l K=1 transposes + out transposes 0-3
        #   in a stolen slot; reciprocal; scale 0-3.
        # part3 (next loop, it==11): out transposes 4-7; scale; DMA out.
        def make_tail(b, xh, po, accs4):
            st = {}

            def part0(l_first=False):
                outu = outs.tile([128, XH], F32, tag="outu")
                nc.vector.tensor_copy(outu[:], po[:])
                out_sb = outs.tile([128, XH], F32, tag="out_sb")
                st.update(outu=outu, out_sb=out_sb)

            def part1(l_first=False):
                steal_l = ps_s.tile([128, XH], F32, tag="ps_s")
                for qd in range(4):
                    for c in range(2):
                        nc.tensor.matmul(
                            steal_l[0:2, bass.ts(c, 512)],
                            ones_bf[:],
                            accs4[qd][:, bass.ts(c, 512)],
                            start=(qd == 0),
                            stop=(qd == 3),
                        )
                l_sb = sb_small.tile([1, XH], F32R, tag="l_sb")
                if l_first:
                    # final tail: ACT is idle, run the l copy there so it
                    # overlaps the DVE outu copy
                    nc.scalar.activation(l_sb[:], steal_l[0:1, :], Act.Identity)
                else:
                    nc.vector.tensor_copy(l_sb[:], steal_l[0:1, :])
                st["l_sb"] = l_sb

            def part2():
                # one ps_s slot: cols 0-511 = transposes 0-3 (bank 0),
                # cols 512-527 = transposed l columns (bank 1).
                steal1 = ps_s.tile([128, 528], F32, tag="ps_s")
                for t in range(8):
                    nc.tensor.matmul(
                        steal1[:, 512 + 2 * t : 512 + 2 * t + 2],
                        st["l_sb"][0:1, bass.ts(t, 128)],
                        ones_r[0:1, 0:2],
                        start=True,
                        stop=True,
                    )
                for t in range(4):
                    nc.tensor.transpose(
                        steal1[:, bass.ts(t, 128)],
                        st["outu"][:, bass.ts(t, 128)],
                        ident[:],
                    )
                rl = sb_small.tile([128, 16], F32, tag="rl")
                nc.vector.reciprocal(rl[:], steal1[:, 512:528])
                for t in range(4):
                    nc.vector.tensor_scalar_mul(
                        st["out_sb"][:, bass.ts(t, 128)],
                        steal1[:, bass.ts(t, 128)],
                        rl[:, 2 * t : 2 * t + 1],
                    )
                st["rl"] = rl

            def part3():
                steal2 = ps_s.tile([128, 512], F32, tag="ps_s")
                for t in range(4):
                    nc.tensor.transpose(
                        steal2[:, bass.ts(t, 128)],
                        st["outu"][:, bass.ts(4 + t, 128)],
                        ident[:],
                    )
                for t in range(4):
                    nc.vector.tensor_scalar_mul(
                        st["out_sb"][:, bass.ts(4 + t, 128)],
                        steal2[:, bass.ts(t, 128)],
                        st["rl"][:, 2 * (4 + t) : 2 * (4 + t) + 1],
                    )
                # out[b] rows [xh*1024,...) as [128p, (8t 128h)], row=t*128+p
                out_view = bass.AP(
                    tensor=out,
                    offset=b * S * H + xh * 8 * 128 * H,
                    ap=[[H, 128], [128 * H, 8], [1, H]],
                )
                nc.sync.dma_start(out_view, st["out_sb"][:])

            return part0, part1, part2, part3

        pending = None  # (part1, part2, part3) of the previous (b, xh)

        # Software-pipelined emission: MM1(j) at iteration j, MM2(j) at
        # iteration j+2, so exp(j) has ~2 iterations of slack before the
        # in-order PE queue reaches its consumer.  The softmax denominator
        # is reduced on DVE: bf16 pair-sums then quad-sums (both at 2x
        # rate), leaving PE only 8 small matmuls per x-half (in the tail).
        def emit_mm1_exp(qr, kr, xh, it, ets):
            pss = ps_s.tile([128, XH], F32, tag="ps_s")
            qj = qr[:, bass.ts(it, 128)]
            for c in range(2):
                nc.tensor.matmul(
                    pss[:, bass.ts(c, 512)],
                    qj,
                    kr[:, bass.ds(xh * XH + c * 512, 512)],
                    start=True,
                    stop=True,
                )
            et = et_pool.tile([128, XH], BF16, tag="et")
            ets[it] = et
            # bias -30 shifts the exp range: overflow now needs a score
            # > 118 instead of 88.7; the shift cancels exactly in the
            # softmax normalization (both numerator and l scale by e^-30)
            nc.scalar.activation(et[:], pss[:], Act.Exp, bias=exp_bias[:])

        seq = [(b, xh) for b in range(BPC) for xh in range(2)]
        heads = {}  # idx -> ets dict with pre-emitted iterations
        for idx, (b, xh) in enumerate(seq):
            qr, kr, vr = qkv_b[b]
            po = ps_o.tile([128, XH], F32)
            ets = heads.pop(idx, {})
            accs2 = {}
            accs4 = {}
            for it in range(NJ + 4):
                if it in (NJ, NJ + 1) and idx + 1 < len(seq):
                    # head of the next (b, xh): keep PE and ACT primed
                    nb, nxh = seq[idx + 1]
                    nqr, nkr, _ = qkv_b[nb]
                    h = heads.setdefault(idx + 1, {})
                    emit_mm1_exp(nqr, nkr, nxh, it - NJ, h)
                if it < NJ and it not in ets:
                    emit_mm1_exp(qr, kr, xh, it, ets)
                # deferred tail of the previous (b, xh): part1 (l-sums)
                # lands in the MM2-free iteration 3, filling the PE while
                # MM2(0) waits for the previous po evacuation
                if pending is not None and it == 3:
                    pending[0]()
                jj = it - 4
                if 0 <= jj < NJ:
                    vj = vr[:, bass.ts(jj, 128)]
                    for c in range(2):
                        nc.tensor.matmul(
                            po[:, bass.ts(c, 512)],
                            vj,
                            ets[jj][:, bass.ts(c, 512)],
                            start=(jj == 0),
                            stop=(jj == NJ - 1),
                        )
                if pending is not None:
                    if it == 7:
                        pending[1]()
                    elif it == 11:
                        pending[2]()
                        pending = None
                # pair-sum p once MM2(2p+1) is emitted (it = 2p+5)
                if it >= 5 and it % 2 == 1 and (it - 5) // 2 < NJ // 2:
                    p = (it - 5) // 2
                    acc2 = a2_pool.tile([128, XH], BF16, tag="acc2")
                    accs2[p] = acc2
                    nc.vector.tensor_add(
                        acc2[:], ets.pop(2 * p)[:], ets.pop(2 * p + 1)[:]
                    )
                # quad-sum q once pairs 2q, 2q+1 exist (it = 4q+7)
                if it >= 7 and (it - 7) % 4 == 0 and (it - 7) // 4 < NJ // 4:
                    qd = (it - 7) // 4
                    acc4 = a4_pool.tile([128, XH], BF16, tag="acc4")
                    accs4[qd] = acc4
                    nc.vector.tensor_add(
                        acc4[:], accs2.pop(2 * qd)[:], accs2.pop(2 * qd + 1)[:]
                    )
                if idx == 0 and BPC > 1 and it == 6:
                    # prefetch next batch
                    qkv_b[1] = emit_loads(1, fine=False)

            part0, part1, part2, part3 = make_tail(b, xh, po, dict(accs4))
            last = idx == len(seq) - 1
            part0(l_first=last)
            if last:
                part1(l_first=True)
                part2()
                part3()
            else:
                pending = (part1, part2, part3)

    _split_waits(nc)
    return nc


_NC_CACHE = None


def _get_nc():
    global _NC_CACHE
    if _NC_CACHE is None:
        _NC_CACHE = _build()
    return _NC_CACHE


def kernel(q: np.ndarray, k: np.ndarray, v: np.ndarray) -> np.ndarray:
    q = np.asarray(q, dtype=np.float32)
    k = np.asarray(k, dtype=np.float32)
    v = np.asarray(v, dtype=np.float32)
    qT = np.ascontiguousarray(q.transpose(0, 2, 1))  # [B, H, S]
    kT = np.ascontiguousarray(k.transpose(0, 2, 1))

    nc = _get_nc()
    in_maps = []
    for c in range(NCORES):
        sl = slice(BPC * c, BPC * (c + 1))
        in_maps.append(
            {
                "qt": np.ascontiguousarray(qT[sl]),
                "kt": np.ascontiguousarray(kT[sl]),
                "v": np.ascontiguousarray(v[sl]),
            }
        )

    trace = bool(int(os.environ.get("ATTN_KERNEL_TRACE", "0")))
    kwargs = {}
    if trace:
        _install_ntff_hook()
        kwargs["trace"] = True
        tmpdir = os.environ.get("ATTN_KERNEL_TRACE_DIR")
        if tmpdir:
            kwargs["tmpdir"] = tmpdir
    try:
        res = run_bass_kernel_spmd(
            nc, in_maps, core_ids=list(range(NCORES)), **kwargs
        )
    except Exception:
        # transient NRT/device hiccups have been observed once; retry
        res = run_bass_kernel_spmd(
            nc, in_maps, core_ids=list(range(NCORES)), **kwargs
        )
    if trace:
        kernel.last_results = res
    out = np.concatenate([res.results[c]["out"] for c in range(NCORES)], axis=0)
    return out.astype(np.float32)


# revision 11
# speedup vs baseline: 1.0001x; 1.0001x over previous
"""Batch-parallel attention kernel for 8 Trainium2 NeuronCores.

Problem: out[b,x,h] = sum_y softmax_y(sum_h' k[b,x,h']*q[b,y,h']) * v[b,y,h]
with q,k,v: [16, 2048, 128] fp32.  This is standard attention with the roles
of q and k swapped (queries = k rows, keys = q rows), no 1/sqrt(H) scale.

Sharding: batch dim (16) across 8 cores (pure data parallel), 2 batches per
core; flash-style x/y block tiling within a core.

Per-core algorithm (per batch, per x-half of 1024 score columns):
  Host supplies qT/kT = q/k transposed to [H, S] so H=128 sits on SBUF
  partitions (v stays natural).  fp32 and f32r are bit-identical, so DMA
  loads land directly in f32r tiles (no DVE cast pass at all).
  For each y-block j (128 rows):
    sT_j[y, x]   = qT_j^T @ kT       (f32r matmuls, N=512, PSUM)
    eT_j         = exp(sT_j - 30)    (ScalarE, PSUM -> SBUF, BF16 out; the
                                      -30 shift widens overflow headroom and
                                      cancels exactly in the normalization)
    outT[h, x]  += v_j^T @ eT_j      (PSUM accumulate over all j; bf16
                                      moving operand, 1 cyc/row)
    acc2(p)      = eT_2p + eT_2p+1   (DVE pair-sums, bf16 at 2x rate)
    acc4(q)      = acc2 + acc2       (DVE quad-sums, bf16 at 2x rate)
  The softmax denominator l = ones^T @ acc4 runs on PE over only 4 tiles
  (8 small matmuls) instead of 18, deferred into the next loop's tail.
  No running-max subtraction is needed: scores are ~N(0, sqrt(128)) and the
  observed max ~84 stays far below the shifted overflow point (118.7).

Scheduling (in-order engine queues make emission order = execution order
per engine):
  - MM1(j) is emitted at iteration j; MM2(j) at iteration j+2, giving
    exp(j) a ~2-iteration window instead of zero slack.
  - ps_s has 3 PSUM slots (6 banks) + outT accumulator (2 banks) = 8.
    The l accumulator PSUM is gone (l-sums borrow a ps_s slot in the tail).
  - The first two MM1/exp of the next (b, xh) are emitted inside the last
    two iterations of the current one, so ACT never drains at boundaries.
  - Each (b, xh)'s tail is deferred into the next loop's iterations 3/7/11
    (l-sums + l copy; l transposes + out transposes 0-3 + scale; out
    transposes 4-7 + scale + store), borrowing ps_s slots so the
    DVE->PE dependency latency hides under real work.
  - A dummy-matmul chain + a dummy Exp at the start warm the PE HAM clock
    gate and preload the ACT table set while the first DMAs run.
"""
import os
import sys
import types
from contextlib import ExitStack

import numpy as np

import concourse.bass as bass
import concourse.tile as tile
from concourse import mybir
from concourse.bass_utils import run_bass_kernel_spmd
from concourse.masks import make_identity

F32 = mybir.dt.float32
F32R = mybir.dt.float32r
BF16 = mybir.dt.bfloat16
Act = mybir.ActivationFunctionType

B, S, H = 16, 2048, 128
NCORES = 8
BPC = B // NCORES  # batches per core
XH = 1024          # x-half width
NJ = S // 128      # y blocks


# ---------------------------------------------------------------------------
# Workaround: this walrus build rejects instructions carrying more than one
# semaphore wait ("Too many sync wait commands", seen on CTRL Drain and S3_LW
# Matmult).  Hoist all-but-one wait of every instruction onto wait-only
# EventSemaphore instructions on the same engine, inserted just before it.
_wsplit_counter = [0]


def _split_waits(nc, max_waits: int = 1):
    for func in nc.m.functions:
        for blk in func.blocks:
            insts = blk.instructions
            i = 0
            while i < len(insts):
                inst = insts[i]
                si = inst.sync_info
                waits = list(si.on_wait) if si is not None else []
                if len(waits) > max_waits:
                    keep = waits[-max_waits:]
                    hoist = waits[:-max_waits]
                    inst.sync_info = mybir.SyncInfo(
                        on_wait=keep, on_update=list(si.on_update)
                    )
                    new_insts = []
                    for w in hoist:
                        _wsplit_counter[0] += 1
                        ev = mybir.InstEventSemaphore(
                            name=f"WSPLIT-{_wsplit_counter[0]}", ins=[], outs=[]
                        )
                        ev.engine = inst.engine
                        ev.sync_info = mybir.SyncInfo(on_wait=[w], on_update=[])
                        new_insts.append(ev)
                    insts[i:i] = new_insts
                    i += len(new_insts)
                i += 1


# NTFF profiling shim: the axon .so supports NRT profiling but the antenv
# glue module is absent in this image; register it so trace=True works.
def _install_ntff_hook():
    if "antenv.axon_hooks" in sys.modules:
        return
    try:
        from trn_agent_boot.trn_boot import _ntff_profile_via_ctypes

        hook = _ntff_profile_via_ctypes("/opt/axon/libaxon_pjrt.so")
    except Exception:
        hook = None
    mod = types.ModuleType("antenv.axon_hooks")
    mod.get_axon_ntff_profile_hook = lambda: hook
    mod.set_axon_ntff_profile_hook = lambda h: None
    sys.modules["antenv.axon_hooks"] = mod


def _build():
    nc = bass.Bass("TRN2", target_bir_lowering=False, debug=False)
    qt = nc.dram_tensor("qt", [BPC, H, S], F32, kind="ExternalInput")
    kt = nc.dram_tensor("kt", [BPC, H, S], F32, kind="ExternalInput")
    v = nc.dram_tensor("v", [BPC, S, H], F32, kind="ExternalInput")
    out = nc.dram_tensor("out", [BPC, S, H], F32, kind="ExternalOutput")

    with tile.TileContext(nc) as tc, ExitStack() as ctx:
        consts = ctx.enter_context(tc.tile_pool(name="consts", bufs=1))
        raw = ctx.enter_context(tc.tile_pool(name="raw", bufs=4))
        qkv = ctx.enter_context(tc.tile_pool(name="qkv", bufs=2))
        et_pool = ctx.enter_context(tc.tile_pool(name="et", bufs=12))
        a2_pool = ctx.enter_context(tc.tile_pool(name="a2", bufs=4))
        a4_pool = ctx.enter_context(tc.tile_pool(name="a4", bufs=6))
        sb_small = ctx.enter_context(tc.tile_pool(name="sb_small", bufs=2))
        outs = ctx.enter_context(tc.tile_pool(name="outs", bufs=2))
        ps_s = ctx.enter_context(tc.tile_pool(name="ps_s", bufs=3, space="PSUM"))
        ps_o = ctx.enter_context(tc.tile_pool(name="ps_o", bufs=1, space="PSUM"))

        ident = consts.tile([128, 128], F32)
        make_identity(nc, ident[:])
        # touch Exp first thing so the ACT table set loads under the DMAs
        warm = consts.tile([128, 2], F32)
        nc.vector.memset(warm[:], 0.0)
        nc.scalar.activation(warm[:], warm[:], Act.Exp)
        exp_bias = consts.tile([128, 1], F32)
        nc.vector.memset(exp_bias[:], -30.0)
        ones_f = consts.tile([128, 2], F32)
        nc.vector.memset(ones_f[:], 1.0)
        ones_r = consts.tile([128, 2], F32R)
        nc.vector.tensor_copy(ones_r[:], ones_f[:])
        ones_bf = consts.tile([128, 2], BF16)
        nc.vector.tensor_copy(ones_bf[:], ones_f[:])
        # dummy matmul chain: keeps the PE busy during the initial DMAs so
        # the HAM clock-gate is at full rate when real matmuls arrive
        warm_z = consts.tile([128, 512], F32, tag="wz")
        nc.vector.memset(warm_z[:], 0.0)
        warm_r = consts.tile([128, 512], F32R)
        nc.vector.tensor_copy(warm_r[:], warm_z[:])
        ps_junk = ps_s.tile([128, XH], F32, tag="ps_s")
        for _ in range(10):
            nc.tensor.matmul(
                ps_junk[:, 0:512], warm_r[:, 0:128], warm_r[:], start=True, stop=True
            )
        junk_sb = consts.tile([128, 2], F32, tag="wjunk")
        nc.vector.tensor_copy(junk_sb[:], ps_junk[:, 0:2])

        def emit_loads(b, fine):
            # DMA straight into f32r tiles (bit-identical to f32), chunked
            # so compute starts early.  First batch uses finer leading
            # chunks to cut the startup serial path.
            qr = qkv.tile([128, S], F32R, tag="qr")
            kr = qkv.tile([128, S], F32R, tag="kr")
            vr = qkv.tile([128, S], BF16, tag="vr")

            def load_k(lo, n):
                if fine:
                    # startup path: sync-engine DMA + DVE cast runs in
                    # parallel with the gpsimd v loads (gpsimd SWDGE
                    # generation is serialized, ~1us per chunk)
                    t = raw.tile([128, n], F32, tag="raw")
                    nc.sync.dma_start(t[:], kt.ap()[b][:, bass.ds(lo, n)])
                    nc.vector.tensor_copy(kr[:, bass.ds(lo, n)], t[:])
                else:
                    nc.gpsimd.dma_start(
                        kr[:, bass.ds(lo, n)], kt.ap()[b][:, bass.ds(lo, n)]
                    )

            def load_q(lo, n):
                if fine:
                    t = raw.tile([128, n], F32, tag="raw")
                    nc.sync.dma_start(t[:], qt.ap()[b][:, bass.ds(lo, n)])
                    nc.vector.tensor_copy(qr[:, bass.ds(lo, n)], t[:])
                else:
                    nc.gpsimd.dma_start(
                        qr[:, bass.ds(lo, n)], qt.ap()[b][:, bass.ds(lo, n)]
                    )

            def load_v(lo, n):
                # v[b] rows [lo, lo+n) presented as [128p, (j 128h)];
                # gpsimd DMA casts f32 -> bf16 in flight (MM2 wants bf16
                # to match the bf16 eT moving operand)
                v_chunk = bass.AP(
                    tensor=v,
                    offset=b * S * H + lo * H,
                    ap=[[H, 128], [128 * H, n // 128], [1, H]],
                )
                nc.gpsimd.dma_start(vr[:, bass.ds(lo, n)], v_chunk)

            if fine:
                load_k(0, 512)
                load_q(0, 256)
                load_v(0, 256)
                load_k(512, 512)
                load_q(256, 768)
                load_v(256, 768)
                load_k(1024, 1024)
                load_q(1024, 1024)
                load_v(1024, 1024)
            else:
                for hc in range(2):
                    load_k(hc * XH, XH)
                    load_q(hc * XH, XH)
                    load_v(hc * XH, XH)
            return qr, kr, vr

        qkv_b = {0: emit_loads(0, fine=True)}

        # Tail work for iteration (b, xh) is deferred into the NEXT
        # iteration's j-loop so its DVE->PE dependency latency hides under
        # real work.  part0 (loop end): evacuate po on DVE.
        # part1 (next loop, it==3): l-sums over the 4 acc4 tiles into a
        #   stolen ps_s slot, l copied to SBUF.
        # part2 (next loop, it==7): l K=1 transposes + out transposes 0-3
        #   in a stolen slot; reciprocal; scale 0-3.
        # part3 (next loop, it==11): out transposes 4-7; scale; DMA out.
        def make_tail(b, xh, po, accs4):
            st = {}

            def part0(l_first=False):
                outu = outs.tile([128, XH], F32, tag="outu")
                nc.vector.tensor_copy(outu[:], po[:])
                out_sb = outs.tile([128, XH], F32, tag="out_sb")
                st.update(outu=outu, out_sb=out_sb)

            def part1(l_first=False):
                steal_l = ps_s.tile([128, XH], F32, tag="ps_s")
                for qd in range(4):
                    for c in range(2):
                        nc.tensor.matmul(
                            steal_l[0:2, bass.ts(c, 512)],
                            ones_bf[:],
                            accs4[qd][:, bass.ts(c, 512)],
                            start=(qd == 0),
                            stop=(qd == 3),
                        )
                l_sb = sb_small.tile([1, XH], F32R, tag="l_sb")
                if l_first:
                    # final tail: ACT is idle, run the l copy there so it
                    # overlaps the DVE outu copy
                    nc.scalar.activation(l_sb[:], steal_l[0:1, :], Act.Identity)
                else:
                    nc.vector.tensor_copy(l_sb[:], steal_l[0:1, :])
                st["l_sb"] = l_sb

            def part2():
                # one ps_s slot: cols 0-511 = transposes 0-3 (bank 0),
                # cols 512-527 = transposed l columns (bank 1).
                steal1 = ps_s.tile([128, 528], F32, tag="ps_s")
                for t in range(8):
                    nc.tensor.matmul(
                        steal1[:, 512 + 2 * t : 512 + 2 * t + 2],
                        st["l_sb"][0:1, bass.ts(t, 128)],
                        ones_r[0:1, 0:2],
                        start=True,
                        stop=True,
                    )
                for t in range(4):
                    nc.tensor.transpose(
                        steal1[:, bass.ts(t, 128)],
                        st["outu"][:, bass.ts(t, 128)],
                        ident[:],
                    )
                rl = sb_small.tile([128, 16], F32, tag="rl")
                nc.vector.reciprocal(rl[:], steal1[:, 512:528])
                for t in range(4):
                    nc.vector.tensor_scalar_mul(
                        st["out_sb"][:, bass.ts(t, 128)],
                        steal1[:, bass.ts(t, 128)],
                        rl[:, 2 * t : 2 * t + 1],
                    )
                st["rl"] = rl

            def part3():
                steal2 = ps_s.tile([128, 512], F32, tag="ps_s")
                for t in range(4):
                    nc.tensor.transpose(
                        steal2[:, bass.ts(t, 128)],
                        st["outu"][:, bass.ts(4 + t, 128)],
                        ident[:],
                    )
                for t in range(4):
                    nc.vector.tensor_scalar_mul(
                        st["out_sb"][:, bass.ts(4 + t, 128)],
                        steal2[:, bass.ts(t, 128)],
                        st["rl"][:, 2 * (4 + t) : 2 * (4 + t) + 1],
                    )
                # out[b] rows [xh*1024,...) as [128p, (8t 128h)], row=t*128+p
                out_view = bass.AP(
                    tensor=out,
                    offset=b * S * H + xh * 8 * 128 * H,
                    ap=[[H, 128], [128 * H, 8], [1, H]],
                )
                nc.sync.dma_start(out_view, st["out_sb"][:])

            return part0, part1, part2, part3

        pending = None  # (part1, part2, part3) of the previous (b, xh)

        # Software-pipelined emission: MM1(j) at iteration j, MM2(j) at
        # iteration j+2, so exp(j) has ~2 iterations of slack before the
        # in-order PE queue reaches its consumer.  The softmax denominator
        # is reduced on DVE: bf16 pair-sums then quad-sums (both at 2x
        # rate), leaving PE only 8 small matmuls per x-half (in the tail).
        def emit_mm1_exp(qr, kr, xh, it, ets):
            pss = ps_s.tile([128, XH], F32, tag="ps_s")
            qj = qr[:, bass.ts(it, 128)]
            for c in range(2):
                nc.tensor.matmul(
                    pss[:, bass.ts(c, 512)],
                    qj,
                    kr[:, bass.ds(xh * XH + c * 512, 512)],
                    start=True,
                    stop=True,
                )
            et = et_pool.tile([128, XH], BF16, tag="et")
            ets[it] = et
            # bias -30 shifts the exp range: overflow now needs a score
            # > 118 instead of 88.7; the shift cancels exactly in the
            # softmax normalization (both numerator and l scale by e^-30)
            nc.scalar.activation(et[:], pss[:], Act.Exp, bias=exp_bias[:])

        seq = [(b, xh) for b in range(BPC) for xh in range(2)]
        heads = {}  # idx -> ets dict with pre-emitted iterations
        for idx, (b, xh) in enumerate(seq):
            qr, kr, vr = qkv_b[b]
            po = ps_o.tile([128, XH], F32)
            ets = heads.pop(idx, {})
            accs2 = {}
            accs4 = {}
            for it in range(NJ + 4):
                if it in (NJ, NJ + 1) and idx + 1 < len(seq):
                    # head of the next (b, xh): keep PE and ACT primed
                    nb, nxh = seq[idx + 1]
                    nqr, nkr, _ = qkv_b[nb]
                    h = heads.setdefault(idx + 1, {})
                    emit_mm1_exp(nqr, nkr, nxh, it - NJ, h)
                if it < NJ and it not in ets:
                    emit_mm1_exp(qr, kr, xh, it, ets)
                # deferred tail of the previous (b, xh): part1 (l-sums)
                # lands in the MM2-free iteration 3, filling the PE while
                # MM2(0) waits for the previous po evacuation
                if pending is not None and it == 3:
                    pending[0]()
                jj = it - 4
                if 0 <= jj < NJ:
                    vj = vr[:, bass.ts(jj, 128)]
                    for c in range(2):
                        nc.tensor.matmul(
                            po[:, bass.ts(c, 512)],
                            vj,
                            ets[jj][:, bass.ts(c, 512)],
                            start=(jj == 0),
                            stop=(jj == NJ - 1),
                        )
                if pending is not None:
                    if it == 7:
                        pending[1]()
                    elif it == 11:
                        pending[2]()
                        pending = None
                # pair-sum p once MM2(2p+1) is emitted (it = 2p+5)
                if it >= 5 and it % 2 == 1 and (it - 5) // 2 < NJ // 2:
                    p = (it - 5) // 2
                    acc2 = a2_pool.tile([128, XH], BF16, tag="acc2")
                    accs2[p] = acc2
                    nc.vector.tensor_add(
                        acc2[:], ets.pop(2 * p)[:], ets.pop(2 * p + 1)[:]
                    )
                # quad-sum q once pairs 2q, 2q+1 exist (it = 4q+7)
                if it >= 7 and (it - 7) % 4 == 0 and (it - 7) // 4 < NJ // 4:
                    qd = (it - 7) // 4
                    acc4 = a4_pool.tile([128, XH], BF16, tag="acc4")
                    accs4[qd] = acc4
                    nc.vector.tensor_add(
                        acc4[:], accs2.pop(2 * qd)[:], accs2.pop(2 * qd + 1)[:]
                    )
                if idx == 1 and BPC > 1 and it == 6:
                    # prefetch next batch; deferred to the second x-half so
                    # the gpsimd burst never contends with the startup DMAs
                    qkv_b[1] = emit_loads(1, fine=False)

            part0, part1, part2, part3 = make_tail(b, xh, po, dict(accs4))
            last = idx == len(seq) - 1
            part0(l_first=last)
            if last:
                part1(l_first=True)
                part2()
                part3()
            else:
                pending = (part1, part2, part3)

    _split_waits(nc)
    return nc


_NC_CACHE = None


def _get_nc():
    global _NC_CACHE
    if _NC_CACHE is None:
        _NC_CACHE = _build()
    return _NC_CACHE


def kernel(q: np.ndarray, k: np.ndarray, v: np.ndarray) -> np.ndarray:
    q = np.asarray(q, dtype=np.float32)
    k = np.asarray(k, dtype=np.float32)
    v = np.asarray(v, dtype=np.float32)
    qT = np.ascontiguousarray(q.transpose(0, 2, 1))  # [B, H, S]
    kT = np.ascontiguousarray(k.transpose(0, 2, 1))

    nc = _get_nc()
    in_maps = []
    for c in range(NCORES):
        sl = slice(BPC * c, BPC * (c + 1))
        in_maps.append(
            {
                "qt": np.ascontiguousarray(qT[sl]),
                "kt": np.ascontiguousarray(kT[sl]),
                "v": np.ascontiguousarray(v[sl]),
            }
        )

    trace = bool(int(os.environ.get("ATTN_KERNEL_TRACE", "0")))
    kwargs = {}
    if trace:
        _install_ntff_hook()
        kwargs["trace"] = True
        tmpdir = os.environ.get("ATTN_KERNEL_TRACE_DIR")
        if tmpdir:
            kwargs["tmpdir"] = tmpdir
    try:
        res = run_bass_kernel_spmd(
            nc, in_maps, core_ids=list(range(NCORES)), **kwargs
        )
    except Exception:
        # transient NRT/device hiccups have been observed once; retry
        res = run_bass_kernel_spmd(
            nc, in_maps, core_ids=list(range(NCORES)), **kwargs
        )
    if trace:
        kernel.last_results = res
    out = np.concatenate([res.results[c]["out"] for c in range(NCORES)], axis=0)
    return out.astype(np.float32)
